# revision 18
# baseline (speedup 1.0000x reference)
"""Trainium2 Bass kernel for nn_DUSPSA (SPSA on f(x)=x0^2+Q*x1^2, 1000 iters).

Per-step SPSA update is linear in x given the Rademacher signs:
    x' = M_k(p) x,  M_k = [[c1_k, -c2_k p],[-c3_k p, c4_k]],  p = d0*d1.
Pair matrices (2 steps) have entries affine in (pE, pO, pE*pO) with host
coefficients; 512 global pairs are combined by a single 9-level doubling
tree batched across all blocks.

Engine split per core (data-parallel over batch across 8 cores):
  DMA    int8 delta bits (host-narrowed 0/1 values), 4.1 MB/core, plus
         per-pair coefficients pre-expanded across batch columns on host
         (a stride-0 broadcast operand drops a DVE op from 2x to 1x, so
         the expansion is bought with idle DMA bandwidth instead)
  Act    int8 bits -> +-1 fp16 signs (affine activation)
  GpSimd sign products pE, pO, r = pE*pO
  Vector fp16 pair-matrix build + one 9-level doubling tree batched over
         all 512 pairs (fp16, f32 final level + apply); fp16 packed
         operands give the 2x DVE rate

Note: dependent DVE op pairs need >=2 intervening ops (RAW pipeline
hazard); op order below preserves that invariant.
"""
import numpy as np

import concourse.bass as bass
import concourse.mybir as mybir
from concourse.bass_utils import run_bass_kernel_spmd

ALPHA, GAMMA, Q = 0.602, 0.101, 8.0
N_CORES = 8
BS = 16384
BPC = BS // N_CORES          # 2048 batch elements per core
P = 128                      # partitions
C = BPC // P                 # 16 batch columns per partition
NIT = 1000
NPAD = 1024
T = 128                      # steps per block
NB = NPAD // T               # 8 blocks
NPAIR = T // 2               # 64 pairs per block
GPAIR = NB * NPAIR           # 512 global pairs
NLEV = 9                     # 512 -> 1
f32 = mybir.dt.float32
f16 = mybir.dt.float16
i8 = mybir.dt.int8
MUL = mybir.AluOpType.mult
ADD = mybir.AluOpType.add
XOR = mybir.AluOpType.logical_xor
ACT_COPY = mybir.ActivationFunctionType.Copy

_CACHED = {}


def _build_nc():
    import contextlib

    nc = bass.Bass("TRN2", target_bir_lowering=False, debug=False)
    # per-partition, per-block layout: [eo(2), c(16), k(64), d(2)] int8
    delta = nc.declare_dram_parameter("delta", [P, NB * T * C * 2], i8, isOutput=False)
    xin = nc.declare_dram_parameter("xin", [P, 2 * C], f32, isOutput=False)
    consts = nc.declare_dram_parameter("consts", [1, NB * 8 * NPAIR * C], f16, isOutput=False)
    yout = nc.declare_dram_parameter("yout", [P, 2 * C], f32, isOutput=True)

    KC = NPAIR * C           # 1024 elems: one (c,k) or (k,c) plane

    stack = contextlib.ExitStack()
    with stack:
        sb = lambda name, shape, dt=f32: stack.enter_context(nc.sbuf_tensor(name, shape, dt))
        d8 = [sb(f"d8_{i}", [P, T * C * 2], i8) for i in range(2)]
        s0 = [sb(f"s0_{i}", [P, 2 * KC], f16) for i in range(2)]     # sign(1-2*d0), [eo][c][k]
        s1 = [sb(f"s1_{i}", [P, 2 * KC], f16) for i in range(2)]
        pE = [sb(f"pE_{i}", [P, KC], f16) for i in range(2)]
        pO = [sb(f"pO_{i}", [P, KC], f16) for i in range(2)]
        cstx = [sb(f"cstx_{i}", [P, 8 * KC], f16) for i in range(3)]
        # pair-major temps (c innermost)
        rr = [sb(f"rr_{i}", [P, KC], f16) for i in range(2)]
        tu, tv, tu2, tv2 = (sb(n, [P, KC], f16) for n in ("tu", "tv", "tu2", "tv2"))
        tw, tw2 = tu, tv   # reuse: tu/tv dead after G01
        dumm = sb("dumm", [P, C], f16)
        # doubling-tree arenas, pair-major (c innermost)
        GA = [sb(f"ga{e}", [P, GPAIR * C], f16) for e in range(4)]          # levels 0,2,4,..
        GB = [sb(f"gb{e}", [P, (GPAIR // 2) * C], f16) for e in range(4)]   # levels 1,3,..
        GF = [sb(f"gf{e}", [P, C], f32) for e in range(4)]                  # final (f32)
        t8 = [sb(f"t8_{i}", [P, (GPAIR // 8) * C], f16) for i in range(8)]
        t8f = [sb(f"t8f_{i}", [P, C], f32) for i in range(8)]
        xt = sb("xt", [P, 2 * C])
        y0, y1, a1, a2, a3, a4 = (sb(n, [P, C]) for n in ("y0", "y1", "a1", "a2", "a3", "a4"))
        out_stage = sb("out_stage", [P, 2 * C])
        dma_sem = stack.enter_context(nc.semaphore("dma"))
        gx_sem = stack.enter_context(nc.semaphore("gx"))      # gpsimd xor done (1/block)
        cv_sem = stack.enter_context(nc.semaphore("cv"))      # act convert done (2/block)
        cx_sem = stack.enter_context(nc.semaphore("cx"))      # cstx dma done (16/block)
        l1_sem = stack.enter_context(nc.semaphore("l1"))      # dve L1 done (1/block)
        done_sem = stack.enter_context(nc.semaphore("done"))
        block = stack.enter_context(nc.Block())

        def kc(ap):     # pair-major plane: [P][k][c]
            return ap.rearrange("p (k c) -> p k c", c=C)

        @block.sync
        def _(sync):
            sync.dma_start(out=xt[:], in_=xin[:]).then_inc(dma_sem, 16)
            sync.dma_start(out=xt[:], in_=xin[:]).then_inc(dma_sem, 16)  # pad to keep counts
            for b in range(NB):
                if b >= 2:
                    sync.wait_ge(cv_sem, 2 * (b - 1))    # act consumed d8[b-2]
                sync.dma_start(
                    out=d8[b % 2][:], in_=delta[:, b * T * C * 2 : (b + 1) * T * C * 2]
                ).then_inc(dma_sem, 16)
            sync.wait_ge(done_sem, 1)
            sync.dma_start(out=yout[:], in_=out_stage[:]).then_inc(dma_sem, 16)

        @block.scalar
        def _(scalar):
            for b in range(NB):
                if b >= 3:
                    scalar.wait_ge(l1_sem, b - 2)        # dve consumed cstx[b-3]
                scalar.dma_start(
                    out=cstx[b % 3][:],
                    in_=consts[0:1, b * 8 * KC : (b + 1) * 8 * KC]
                    .partition_broadcast(P).squeeze(1),
                ).then_inc(cx_sem, 16)
                scalar.wait_ge(dma_sem, 32 + 16 * (b + 1))
                if b >= 2:
                    scalar.wait_ge(gx_sem, 2 * (b - 1))  # gpsimd consumed s0/s1[b-2]
                d = d8[b % 2][:].rearrange("p (e k c d) -> p e k c d", e=2, k=NPAIR, c=C)
                so = s0[b % 2][:].rearrange("p (e k c) -> p e k c", e=2, k=NPAIR)
                s1o = s1[b % 2][:].rearrange("p (e k c) -> p e k c", e=2, k=NPAIR)
                scalar.activation(so, d[:, :, :, :, 0], ACT_COPY, bias=1.0, scale=-2.0)
                scalar.activation(
                    s1o, d[:, :, :, :, 1], ACT_COPY, bias=1.0, scale=-2.0
                ).then_inc(cv_sem, 2)

        @block.gpsimd
        def _(gpsimd):
            for b in range(NB):
                gpsimd.wait_ge(cv_sem, 2 * (b + 1))
                if b >= 2:
                    gpsimd.wait_ge(l1_sem, b - 1)        # dve consumed pE/pO/rr[b-2]
                a = s0[b % 2][:]
                bb = s1[b % 2][:]
                gpsimd.tensor_tensor(
                    pE[b % 2][:], a[:, 0:KC], bb[:, 0:KC], MUL
                ).then_inc(gx_sem, 1)
                gpsimd.tensor_tensor(
                    pO[b % 2][:], a[:, KC : 2 * KC], bb[:, KC : 2 * KC], MUL
                ).then_inc(gx_sem, 1)
                gpsimd.tensor_tensor(
                    rr[b % 2][:], pE[b % 2][:], pO[b % 2][:], MUL
                ).then_inc(gx_sem, 1)

        @block.vector
        def _(vector):
            vector.wait_ge(dma_sem, 32)
            vector.tensor_scalar(y0[:], xt[:, 0 : 2 * C : 2], 20.0, -10.0, MUL, ADD)
            vector.tensor_scalar(y1[:], xt[:, 1 : 2 * C : 2], 20.0, -10.0, MUL, ADD)

            for b in range(NB):
                vector.wait_ge(gx_sem, 3 * (b + 1))
                vector.wait_ge(cx_sem, 16 * (b + 1))
                cpE, cpO = kc(pE[b % 2][:]), kc(pO[b % 2][:])
                crr = kc(rr[b % 2][:])
                cx = lambda idx: kc(cstx[b % 3][:, idx * KC : (idx + 1) * KC])
                sl = slice(b * NPAIR * C, (b + 1) * NPAIR * C)
                G0 = [kc(GA[e][:, sl]) for e in range(4)]
                vector.tensor_tensor(kc(tu[:]), cpE, cx(0), MUL)
                vector.tensor_tensor(kc(tv[:]), cpO, cx(1), MUL)
                vector.tensor_tensor(kc(tu2[:]), cpE, cx(2), MUL)
                vector.tensor_tensor(kc(tv2[:]), cpO, cx(3), MUL)
                vector.tensor_tensor(G0[1], kc(tu[:]), kc(tv[:]), ADD)
                vector.tensor_tensor(G0[2], kc(tu2[:]), kc(tv2[:]), ADD)
                vector.tensor_tensor(kc(tw[:]), crr, cx(4), MUL)
                vector.tensor_tensor(kc(tw2[:]), crr, cx(6), MUL)
                vector.tensor_tensor(G0[0], kc(tw[:]), cx(5), ADD)
                vector.tensor_tensor(
                    G0[3], kc(tw2[:]), cx(7), ADD,
                ).then_inc(l1_sem, 1)

            # ---- 9 doubling levels, batched across all blocks ----
            # level 0 (m=256) runs in two free-dim chunks so t8 temps fit SBUF
            arenas = [GA, GB]
            m = GPAIR
            p3 = lambda ap: ap.rearrange("p (k c) -> p k c", c=C)
            for l in range(NLEV):
                m //= 2
                Gp = arenas[l % 2]
                Gn = GF if l == NLEV - 1 else arenas[(l + 1) % 2]
                last = l == NLEV - 1
                nch = 4 if l == 0 else (2 if l == 1 else 1)
                mc = m // nch
                for ch in range(nch):
                    src = slice(ch * 2 * mc * C, (ch + 1) * 2 * mc * C)
                    dst = slice(ch * mc * C, (ch + 1) * mc * C)
                    E = [p3(Gp[e][:, src])[:, 0 : 2 * mc : 2, :] for e in range(4)]
                    F = [p3(Gp[e][:, src])[:, 1 : 2 * mc : 2, :] for e in range(4)]
                    tsrc = t8f if last else t8
                    t1, t2, t3, t4, t5, t6, t7, t8v = [
                        p3(t[:, 0 : mc * C]) for t in tsrc
                    ]
                    O = [p3(Gn[e][:, dst]) for e in range(4)]
                    vector.tensor_tensor(t2, F[1], E[2], MUL)
                    vector.tensor_tensor(t7, F[2], E[1], MUL)
                    vector.tensor_tensor(t1, F[0], E[0], MUL)
                    vector.tensor_tensor(t5, F[2], E[0], MUL)
                    vector.tensor_tensor(t3, F[0], E[1], MUL)
                    vector.tensor_tensor(t8v, F[3], E[3], MUL)
                    vector.tensor_tensor(t4, F[1], E[3], MUL)
                    vector.tensor_tensor(t6, F[3], E[2], MUL)
                    vector.tensor_tensor(O[0], t1, t2, ADD)
                    vector.tensor_tensor(O[1], t3, t4, ADD)
                    vector.tensor_tensor(O[2], t5, t6, ADD)
                    vector.tensor_tensor(O[3], t7, t8v, ADD)

            # ---- apply total matrix to scaled x0 ----
            vector.tensor_tensor(a1[:], GF[0][:], y0[:], MUL)
            vector.tensor_tensor(a2[:], GF[1][:], y1[:], MUL)
            vector.tensor_tensor(a3[:], GF[2][:], y0[:], MUL)
            vector.tensor_tensor(a4[:], GF[3][:], y1[:], MUL)
            vector.tensor_tensor(out_stage[:, 0:C], a1[:], a2[:], ADD)
            vector.tensor_copy(dumm[:], a1[:])  # RAW hazard spacer
            vector.tensor_tensor(
                out_stage[:, C : 2 * C], a3[:], a4[:], ADD
            ).then_inc(done_sem, 1)

    return nc


def _host_constants(a, c, num_itr):
    n = int(num_itr)
    A = int(np.floor(0.1 * n))
    k = np.arange(1, n + 1, dtype=np.float64)
    ak = a.astype(np.float64) / (k + 1.0 + A) ** ALPHA
    c1 = 1.0 - 2.0 * ak
    c4 = 1.0 - 2.0 * ak * Q
    c2 = 2.0 * ak * Q
    c3 = 2.0 * ak
    pad = NPAD - n
    c1 = np.concatenate([c1, np.ones(pad)])
    c4 = np.concatenate([c4, np.ones(pad)])
    c2 = np.concatenate([c2, np.zeros(pad)])
    c3 = np.concatenate([c3, np.zeros(pad)])
    e = np.arange(0, NPAD, 2)
    o = e + 1
    # G = M_o @ M_e, M = [[c1, -c2 p],[-c3 p, c4]]
    g1 = -(c1[o] * c2[e])      # * pE  -> G01
    g2 = -(c2[o] * c4[e])      # * pO
    h1 = -(c4[o] * c3[e])      # * pE  -> G10
    h2 = -(c3[o] * c1[e])      # * pO
    beta = c2[o] * c3[e]       # * r   -> G00
    alpha = c1[o] * c1[e]
    beta2 = c3[o] * c2[e]      # * r   -> G11
    alpha2 = c4[o] * c4[e]
    rows = np.stack([g1, g2, h1, h2, beta, alpha, beta2, alpha2], axis=0).astype(np.float16)
    out = np.zeros((NB, 8, NPAIR), np.float16)
    for b in range(NB):
        out[b] = rows[:, b * NPAIR : (b + 1) * NPAIR]
    out = np.repeat(out.reshape(NB, 8, NPAIR, 1), C, axis=3)
    return np.ascontiguousarray(out).reshape(1, -1)


def _prep_in_maps(X0, a, c, delta_bits, n):
    consts = _host_constants(a, c, n)
    dpad = np.zeros((NPAD, BS, 2), np.int8)
    dpad[:n] = delta_bits
    in_maps = []
    for ci in range(N_CORES):
        sl = slice(ci * BPC, (ci + 1) * BPC)
        # [step, bpc, d] -> [b, k, eo, p, c, d] -> [p, b, eo, k, c, d]
        d = dpad[:, sl, :].reshape(NB, NPAIR, 2, P, C, 2).transpose(3, 0, 2, 1, 4, 5)
        d = np.ascontiguousarray(d).reshape(P, NB * T * C * 2)
        x = np.ascontiguousarray(X0[sl].reshape(P, 2 * C))
        in_maps.append({"delta": d, "xin": x, "consts": consts})
    return in_maps


def _gather(results):
    out = np.empty((BS, 2), np.float32)
    for ci in range(N_CORES):
        y = results[ci]["yout"]
        sl = slice(ci * BPC, (ci + 1) * BPC)
        out[sl, 0] = y[:, 0:C].reshape(BPC)
        out[sl, 1] = y[:, C : 2 * C].reshape(BPC)
    return out


def kernel(X0, a, c, delta_bits, num_itr, **run_kwargs):
    X0 = np.ascontiguousarray(np.asarray(X0, np.float32))
    a = np.asarray(a, np.float32)
    c = np.asarray(c, np.float32)
    delta_bits = np.asarray(delta_bits, np.int32)
    n = int(num_itr)
    assert X0.shape == (BS, 2) and delta_bits.shape == (n, BS, 2) and n == NIT

    if "nc" not in _CACHED:
        _CACHED["nc"] = _build_nc()
    nc = _CACHED["nc"]

    in_maps = _prep_in_maps(X0, a, c, delta_bits, n)
    res = run_bass_kernel_spmd(nc, in_maps, core_ids=list(range(N_CORES)), **run_kwargs)
    out = _gather(res.results)
    if run_kwargs:
        return out, res
    return out


if __name__ == "__main__":
    rng = np.random.default_rng(0)
    X0 = rng.random((BS, 2), dtype=np.float32)
    a = np.full((NIT,), 0.01, np.float32)
    c = np.full((NIT,), 0.01, np.float32)
    db = rng.integers(0, 2, size=(NIT, BS, 2), dtype=np.int32)
    out = kernel(X0=X0, a=a, c=c, delta_bits=db, num_itr=NIT)
    print("kernel ran, out:", out.shape, out.dtype, float(np.abs(out).max()))
